# revision 8
# baseline (speedup 1.0000x reference)
"""NoisyTopKRouter (eval) for 8x TRN2 NeuronCores.

reference: h = gelu(x @ W1 + b1, exact); logits = h @ W2 + b2
           top8 -> softmax over selected -> scatter to [N, E]
           load_balance_loss = E * sum(freq * avg_prob)

Strategy (data-parallel over tokens, 2048 tokens/core):
  - Matmuls in fp16 hi/lo 3-pass split (xh@wh + xl@wh + xh@wl), fp32 PSUM
    accumulation -> ~2^-22-grade logits (max |dlogit| ~2e-6 on this data).
  - gelu via ACT spline (erf-based fallback selectable), top-8 via
    nc.vector.max / max_index, routing weights = full-softmax masked to the
    top-8 and renormalized (mathematically identical to softmax(top_vals)).
  - Per-core partial expert counts + prob sums via ones-matmul; host combines
    across cores for the scalar load-balance loss.
  - Device also emits top-8 values + 9th-largest value per token; the host
    recomputes the few tokens whose top-k decision margin is below a safety
    threshold (~0.3% worst case) in fp64 to make the top-k selection exact.
"""

import os
import sys

import numpy as np

for _p in ("/opt/trn_rl_repo", "/root/.axon_site/_ro/trn_rl_repo"):
    if os.path.isdir(_p) and _p not in sys.path:
        sys.path.insert(0, _p)

import concourse.bass as bass
import concourse.bacc as bacc
import concourse.mybir as mybir
from concourse.tile import TileContext
from concourse.bass_utils import run_bass_kernel_spmd

F32 = mybir.dt.float32
F16 = mybir.dt.float16
U32 = mybir.dt.uint32
AF = mybir.ActivationFunctionType
ALU = mybir.AluOpType

N_CORES = 8
N = 16384
T = N // N_CORES          # tokens per core
D = 2048
E = 64
TOPK = 8
KO = D // 128             # contraction tiles
MO = D // 128             # dout tiles
CHUNK = 512               # token chunk in phase A
CHUNKS = T // CHUNK
TT = T // 128             # token tiles per core (16)

USE_ERF = False           # False: ACT Gelu spline; True: erf-based exact gelu
# tokens whose minimum top-k decision gap is below this are recomputed on host
FIXUP_THRESHOLD = 2e-5


def build_nc():
    nc = bacc.Bacc("TRN2")
    d = {}

    def din(name, shape, dt):
        d[name] = nc.dram_tensor(name, shape, dt, kind="ExternalInput")

    def dout(name, shape, dt):
        d[name] = nc.dram_tensor(name, shape, dt, kind="ExternalOutput")

    din("xh", [128, KO, T], F16)      # x_shard^T hi  (p, ko, t)
    din("xl", [128, KO, T], F16)      # x_shard^T lo
    din("w1h", [128, MO, KO, 128], F16)  # (p=k_in_tile, m, ko, j=dout_in_tile)
    din("w1l", [128, MO, KO, 128], F16)
    din("w2h", [128, KO, E], F16)     # (p, ko, e); here "ko" indexes dout tiles
    din("w2l", [128, KO, E], F16)
    din("b1t", [128, MO], F32)        # b1 arranged per-partition: [p, m]
    din("b1s", [128, MO], F32)        # b1 / sqrt(2) (erf path)
    din("b2r", [128, E], F32)         # b2 replicated across partitions
    din("ones", [128, 1], F16)

    dout("rw", [128, TT, E], F32)     # routing weights (p, t, e)
    dout("ti", [128, TT, TOPK], U32)  # top-8 indices
    dout("tv", [128, TT, TOPK], F32)  # top-8 values (for host fixup margins)
    dout("l9", [128, TT, 1], F32)     # 9th largest logit
    dout("cnt", [1, E], F32)          # partial expert counts
    dout("ps", [1, E], F32)           # partial prob sums

    with TileContext(nc) as tc:
        with (
            tc.tile_pool(name="const", bufs=1) as cp,
            tc.tile_pool(name="xchunk", bufs=2) as xp,
            tc.tile_pool(name="w1t", bufs=3) as w1p,
            tc.tile_pool(name="hbuf", bufs=2) as hp,
            tc.tile_pool(name="tmp", bufs=3) as tp,
            tc.tile_pool(name="epi", bufs=2) as ep,
            tc.tile_pool(name="psA", bufs=3, space="PSUM") as psA,
            tc.tile_pool(name="psB", bufs=2, space="PSUM") as psB,
            tc.tile_pool(name="psP", bufs=1, space="PSUM") as psP,
        ):
            w2h = cp.tile([128, KO, E], F16)
            w2l = cp.tile([128, KO, E], F16)
            b1t = cp.tile([128, MO], F32)
            b1s = cp.tile([128, MO], F32)
            b2r = cp.tile([128, E], F32)
            ones = cp.tile([128, 1], F16)
            for t_, n_ in [(w2h, "w2h"), (w2l, "w2l"), (b1t, "b1t"), (b1s, "b1s"),
                           (b2r, "b2r"), (ones, "ones")]:
                nc.sync.dma_start(t_[:], d[n_][:])

            cnt_acc = cp.tile([1, E], F32)
            ps_acc = cp.tile([1, E], F32)
            nc.vector.memset(cnt_acc[:], 0.0)
            nc.vector.memset(ps_acc[:], 0.0)

            for c in range(CHUNKS):
                tok = slice(c * CHUNK, (c + 1) * CHUNK)
                xh_c = xp.tile([128, KO, CHUNK], F16, tag="xh")
                xl_c = xp.tile([128, KO, CHUNK], F16, tag="xl")
                nc.sync.dma_start(xh_c[:], d["xh"][:, :, tok])
                nc.sync.dma_start(xl_c[:], d["xl"][:, :, tok])
                # h (post-gelu) for this chunk, fp16 hi/lo, dout on partitions
                hh_c = hp.tile([128, MO, CHUNK], F16, tag="hh")
                hl_c = hp.tile([128, MO, CHUNK], F16, tag="hl")

                with nc.named_scope(f"phaseA_c{c}"):
                    for m in range(MO):
                        w1h_m = w1p.tile([128, KO, 128], F16, tag="w1h")
                        w1l_m = w1p.tile([128, KO, 128], F16, tag="w1l")
                        nc.sync.dma_start(w1h_m[:], d["w1h"][:, m])
                        nc.sync.dma_start(w1l_m[:], d["w1l"][:, m])
                        pa = psA.tile([128, CHUNK], F32, tag="pa")
                        i = 0
                        for lhs, rhs in ((w1h_m, xh_c), (w1h_m, xl_c),
                                         (w1l_m, xh_c)):
                            for k in range(KO):
                                nc.tensor.matmul(pa[:], lhs[:, k, :], rhs[:, k, :],
                                                 start=(i == 0),
                                                 stop=(i == 3 * KO - 1))
                                i += 1
                        # epilogue: h = gelu(pre + b1) ; split into fp16 hi/lo
                        hf = tp.tile([128, CHUNK], F32, tag="hf")
                        if USE_ERF:
                            # erf((pre+b1)/sqrt2) = erf(pre*(1/sqrt2) + b1/sqrt2)
                            erf_t = tp.tile([128, CHUNK], F32, tag="erf")
                            nc.scalar.activation(erf_t[:], pa[:], AF.Erf,
                                                 scale=0.7071067811865476,
                                                 bias=b1s[:, m:m + 1])
                            pre = tp.tile([128, CHUNK], F32, tag="pre")
                            nc.vector.tensor_scalar(pre[:], pa[:], b1t[:, m:m + 1],
                                                    None, ALU.add)
                            # hf = pre*(1+erf) = 2*gelu; 0.5 folded into W2 on host
                            nc.vector.scalar_tensor_tensor(hf[:], erf_t[:], 1.0,
                                                           pre[:], ALU.add,
                                                           ALU.mult)
                        else:
                            nc.scalar.activation(hf[:], pa[:], AF.Gelu,
                                                 bias=b1t[:, m:m + 1])
                        nc.vector.tensor_copy(hh_c[:, m, :], hf[:])
                        nc.vector.tensor_tensor(hl_c[:, m, :], hf[:], hh_c[:, m, :],
                                                ALU.subtract)

                with nc.named_scope(f"phaseB_c{c}"):
                    for t in range(CHUNK // 128):
                        tg = c * (CHUNK // 128) + t   # global token tile
                        tcols = slice(t * 128, (t + 1) * 128)
                        pb = psB.tile([128, E], F32, tag="pb")
                        i = 0
                        for lhs, rhs in ((hh_c, w2h), (hl_c, w2h), (hh_c, w2l)):
                            for k in range(KO):
                                nc.tensor.matmul(pb[:], lhs[:, k, tcols],
                                                 rhs[:, k, :],
                                                 start=(i == 0),
                                                 stop=(i == 3 * KO - 1))
                                i += 1
                        lg = ep.tile([128, E], F32, tag="lg")
                        nc.vector.tensor_tensor(lg[:], pb[:], b2r[:], ALU.add)
                        tv = ep.tile([128, TOPK], F32, tag="tv")
                        ti = ep.tile([128, TOPK], U32, tag="ti")
                        nc.vector.max(out=tv[:], in_=lg[:])
                        nc.vector.max_index(out=ti[:], in_max=tv[:], in_values=lg[:])
                        pe_un = ep.tile([128, E], F32, tag="pe_un")
                        zsum = ep.tile([128, 1], F32, tag="zsum")
                        nc.scalar.activation(pe_un[:], lg[:], AF.Exp,
                                             accum_out=zsum[:])
                        mask = ep.tile([128, E], F16, tag="mask")
                        nc.vector.tensor_scalar(mask[:], lg[:], tv[:, 7:8], None,
                                                ALU.is_ge)
                        rw_un = ep.tile([128, E], F32, tag="rw_un")
                        ssum = ep.tile([128, 1], F32, tag="ssum")
                        nc.vector.scalar_tensor_tensor(rw_un[:], mask[:], 0.0,
                                                       pe_un[:], ALU.bypass,
                                                       ALU.mult,
                                                       accum_out=ssum[:])
                        sinv = ep.tile([128, 1], F32, tag="sinv")
                        zinv = ep.tile([128, 1], F32, tag="zinv")
                        nc.vector.reciprocal(sinv[:], ssum[:])
                        nc.vector.reciprocal(zinv[:], zsum[:])
                        rw_t = ep.tile([128, E], F32, tag="rw_t")
                        nc.vector.tensor_scalar(rw_t[:], rw_un[:], sinv[:], None,
                                                ALU.mult)
                        probs16 = ep.tile([128, E], F16, tag="probs16")
                        nc.vector.tensor_scalar(probs16[:], pe_un[:], zinv[:], None,
                                                ALU.mult)
                        # 9th largest = max over non-top-8
                        l9m = ep.tile([128, E], F32, tag="l9m")
                        nc.vector.scalar_tensor_tensor(l9m[:], mask[:], -1e30,
                                                       lg[:], ALU.mult, ALU.add)
                        l9v = ep.tile([128, 1], F32, tag="l9v")
                        nc.vector.tensor_reduce(l9v[:], l9m[:],
                                                mybir.AxisListType.X, ALU.max)
                        # partial counts / prob sums
                        pc = psP.tile([1, E], F32, tag="pc")
                        nc.tensor.matmul(pc[:], ones[:], mask[:], start=True,
                                         stop=True)
                        nc.vector.tensor_tensor(cnt_acc[:], cnt_acc[:], pc[:],
                                                ALU.add)
                        pp = psP.tile([1, E], F32, tag="pp")
                        nc.tensor.matmul(pp[:], ones[:], probs16[:], start=True,
                                         stop=True)
                        nc.vector.tensor_tensor(ps_acc[:], ps_acc[:], pp[:],
                                                ALU.add)
                        # outputs
                        nc.sync.dma_start(d["rw"][:, tg, :], rw_t[:])
                        nc.sync.dma_start(d["ti"][:, tg, :], ti[:])
                        nc.sync.dma_start(d["tv"][:, tg, :], tv[:])
                        nc.sync.dma_start(d["l9"][:, tg, :], l9v[:])

            nc.sync.dma_start(d["cnt"][:], cnt_acc[:])
            nc.sync.dma_start(d["ps"][:], ps_acc[:])
    return nc


_NC = None


def _get_nc():
    global _NC
    if _NC is None:
        _NC = build_nc()
        _NC.finalize()
    return _NC


def _split16(a):
    hi = a.astype(np.float16)
    lo = (a - hi.astype(np.float32)).astype(np.float16)
    return hi, lo


def kernel(x, W1, b1, W2, b2):
    x = np.asarray(x, np.float32)
    W1 = np.asarray(W1, np.float32)
    b1 = np.asarray(b1, np.float32)
    W2 = np.asarray(W2, np.float32)
    b2 = np.asarray(b2, np.float32)

    W2_eff = (0.5 * W2) if USE_ERF else W2

    # ---- host-side prep (shared across cores) ----
    w1h16, w1l16 = _split16(W1)
    # [p, m, ko, j]: element = W1[ko*128+p, m*128+j]
    def w1_layout(w):
        return np.ascontiguousarray(
            w.reshape(KO, 128, MO, 128).transpose(1, 2, 0, 3))
    w1h_l = w1_layout(w1h16)
    w1l_l = w1_layout(w1l16)
    w2h16, w2l16 = _split16(W2_eff)
    w2_l = lambda w: np.ascontiguousarray(w.reshape(KO, 128, E).transpose(1, 0, 2))
    w2h_lo = w2_l(w2h16)
    w2l_lo = w2_l(w2l16)
    b1t = np.ascontiguousarray(b1.reshape(MO, 128).T)          # [p, m]
    b1s = np.ascontiguousarray(b1t / np.sqrt(2.0)).astype(np.float32)
    b2r = np.ascontiguousarray(np.broadcast_to(b2, (128, E)))  # replicated
    ones = np.ones((128, 1), np.float16)

    in_maps = []
    for c in range(N_CORES):
        xs = x[c * T:(c + 1) * T]                      # [T, D]
        xT = np.ascontiguousarray(xs.T)                # [D, T]
        xh16, xl16 = _split16(xT)
        lay = lambda a: np.ascontiguousarray(
            a.reshape(KO, 128, T).transpose(1, 0, 2))  # [p, ko, t]
        in_maps.append({
            "xh": lay(xh16), "xl": lay(xl16),
            "w1h": w1h_l, "w1l": w1l_l,
            "w2h": w2h_lo, "w2l": w2l_lo,
            "b1t": b1t, "b1s": b1s, "b2r": b2r, "ones": ones,
        })

    nc = _get_nc()
    res = run_bass_kernel_spmd(nc, in_maps, core_ids=list(range(N_CORES)))
    kernel.last_result = res

    # ---- gather ----
    rw = np.empty((N, E), np.float32)
    ti = np.empty((N, TOPK), np.int64)
    tv = np.empty((N, TOPK), np.float32)
    l9 = np.empty((N,), np.float32)
    cnt = np.zeros(E, np.float64)
    psum = np.zeros(E, np.float64)
    for c, r in enumerate(res.results):
        sl = slice(c * T, (c + 1) * T)
        rw[sl] = r["rw"].transpose(1, 0, 2).reshape(T, E)
        ti[sl] = r["ti"].transpose(1, 0, 2).reshape(T, TOPK).astype(np.int64)
        tv[sl] = r["tv"].transpose(1, 0, 2).reshape(T, TOPK)
        l9[sl] = r["l9"].transpose(1, 0, 2).reshape(T)
        cnt += r["cnt"][0].astype(np.float64)
        psum += r["ps"][0].astype(np.float64)

    # ---- host fixup of margin-risky tokens (exact fp64 recompute) ----
    gaps = np.empty((N, TOPK), np.float32)
    gaps[:, :7] = tv[:, :7] - tv[:, 1:]
    gaps[:, 7] = tv[:, 7] - l9
    risky = np.where(gaps.min(axis=1) < FIXUP_THRESHOLD)[0]
    if risky.size:
        from scipy.special import erf as _erf
        xr = x[risky].astype(np.float64)
        pre = xr @ W1.astype(np.float64) + b1.astype(np.float64)
        h = pre * 0.5 * (1.0 + _erf(pre / np.sqrt(2.0)))
        lgr = h @ W2.astype(np.float64) + b2.astype(np.float64)
        order = np.argsort(-lgr, axis=1, kind="stable")[:, :TOPK]
        w = np.exp(lgr[np.arange(risky.size)[:, None], order])
        w = w / w.sum(1, keepdims=True)
        for j, tok in enumerate(risky):
            old_set = ti[tok]
            new_set = order[j]
            if not np.array_equal(np.sort(old_set), np.sort(new_set)):
                np.subtract.at(cnt, old_set, 1.0)
                np.add.at(cnt, new_set, 1.0)
            ti[tok] = new_set
            row = np.zeros(E, np.float32)
            row[new_set] = w[j].astype(np.float32)
            rw[tok] = row
    kernel.n_fixed = int(risky.size)

    freq = cnt / N
    avg_prob = psum / N
    loss = np.float32(E * np.sum(freq * avg_prob))
    return rw, ti.astype(np.int32), loss
